# revision 33
# baseline (speedup 1.0000x reference)
"""Trainium2 Bass kernel for causal multi-head attention.

Problem: B=2, S=2048, D=1024, H=16 heads (DH=64), causal, fp32 reference.

Sharding over 8 NeuronCores: core c handles batch b = c//4 and head group
g = c%4 (4 heads each).  Wq/Wk/Wv are split column-wise (by output head),
Wo row-wise; per-core partial outputs are summed on the host (row-parallel
Wo => partial sums), then bo is added.

Per-core device kernel (matmul operands bf16, fp32 PSUM accumulation):
  qT/kT = W @ xT            (64, 2048) per head, head pairs stacked on 128 parts
  v     = x @ WvT           (2048, 256)
  S^T   = k q^T             scores transposed: (s_k, s_q) tiles; diagonal
                            blocks shrunk to the causally-valid q range
  P     = exp(0.125 * S^T)  ScalarE (only exp runs there); the 128-wide
                            triangle at the diagonal masked via DVE mul
  ctx^T = v^T P             accumulated over k tiles in PSUM
  sums: DVE pre-adds each (kt-1, kt) exp-tile pair, then one onescol
        matmul per head per PAIR accumulates into two 32-row strips
        (rows 0 / 32 of m_ps), halving the denominator's PE cost
  denominators: single (33, 512) bf16 evac -> selb broadcast matmul ->
                DVE reciprocal on the broadcast (128, 512) -> DVE
                normalize multiply, riding the next q tile's k loop
  out   = ctx @ WoT         (2048, 1024) fp32 partial, riding the next
                            loop; PSUM->SBUF evac on the idle GpSimd

All non-attention work (projections of the next q tile, output projection
of the previous one, the normalize chain) is queued as "riders" that are
drained at a fixed cadence inside the attention k loops so PE/DVE/ACT all
stay busy; ScalarE's exp paces the steady state.
"""

import sys

sys.path.insert(0, "/opt/trn_rl_repo")

import numpy as np
import ml_dtypes

import concourse.bass as bass
import concourse.bacc as bacc
import concourse.mybir as mybir
import concourse.tile as tile
from concourse.bass_utils import run_bass_kernel_spmd

BF16 = mybir.dt.bfloat16
F32 = mybir.dt.float32
AF = mybir.ActivationFunctionType

B, S, D, H = 2, 2048, 1024, 16
DH = D // H            # 64
NCORES = 8
NH = 4                 # heads per core
DL = NH * DH           # 256 local head dims per core
KD = D // 128          # 8 contraction chunks for projections
SQ = S // 512          # 4 q tiles of 512
ST = S // 128          # 16 s tiles of 128
SCALE = DH ** -0.5     # 0.125
LAG = 3                # ctx trails scores by LAG k-tiles

_NC = None

TRACE = False
LAST_RESULTS = None
DEBUG = False


def _build_nc():
    nc = bacc.Bacc("TRN2", target_bir_lowering=False, debug=False,
                   num_devices=NCORES)

    xT_d = nc.dram_tensor("xT", [D, S], BF16, kind="ExternalInput")
    wq_d = nc.dram_tensor("wqT", [D, DL], BF16, kind="ExternalInput")
    wk_d = nc.dram_tensor("wkT", [D, DL], BF16, kind="ExternalInput")
    wv_d = nc.dram_tensor("wvT", [D, DL], BF16, kind="ExternalInput")
    wo_d = nc.dram_tensor("woT", [DL, D], BF16, kind="ExternalInput")
    # [128, 2, 128] lower-triangle (q >= k) mask, duplicated per head half
    tri_d = nc.dram_tensor("tri", [128, 256], BF16, kind="ExternalInput")
    # ones in column 0, zeros elsewhere (M=32 so strip rows are zero-filled)
    onescol_d = nc.dram_tensor("onescol", [128, 32], BF16, kind="ExternalInput")
    # denominator broadcast selector: row 0 -> ones at cols [0,64),
    # row 32 -> ones at cols [64,128); all other rows zero
    selb_d = nc.dram_tensor("selb", [33, 128], BF16, kind="ExternalInput")
    out_d = nc.dram_tensor("out", [S, D], F32, kind="ExternalOutput")
    dbg = {}
    if DEBUG:
        for j in (1, 2):
            for p in (0, 1):
                dbg[f"q4_{j}_{p}"] = nc.dram_tensor(
                    f"dbg_q4_{j}_{p}", [128, 512], F32, kind="ExternalOutput")
                dbg[f"inv2_{j}_{p}"] = nc.dram_tensor(
                    f"dbg_inv2_{j}_{p}", [2, 512], F32, kind="ExternalOutput")
                dbg[f"invb_{j}_{p}"] = nc.dram_tensor(
                    f"dbg_invb_{j}_{p}", [128, 512], F32, kind="ExternalOutput")
                dbg[f"ctxT_{j}_{p}"] = nc.dram_tensor(
                    f"dbg_ctxT_{j}_{p}", [128, 512], BF16, kind="ExternalOutput")
                dbg[f"ex_{j}_{p}"] = nc.dram_tensor(
                    f"dbg_ex_{j}_{p}", [128, 1024], BF16, kind="ExternalOutput")

    with tile.TileContext(nc) as tc:
        with (
            tc.tile_pool(name="consts", bufs=1) as consts,
            tc.tile_pool(name="xpool", bufs=1) as xpool,
            tc.tile_pool(name="qkpool", bufs=1) as qkpool,
            tc.tile_pool(name="vpool", bufs=1) as vpool,
            tc.tile_pool(name="cpool", bufs=1) as cpool,
            tc.tile_pool(name="exppool", bufs=8) as exppool,
            tc.tile_pool(name="addpool", bufs=3) as addpool,
            tc.tile_pool(name="accpool", bufs=2) as accpool,
            tc.tile_pool(name="smallpool", bufs=2) as smallpool,
            tc.tile_pool(name="outpool", bufs=6) as outpool,
            tc.tile_pool(name="spsum", bufs=2, space="PSUM") as spsum,
            tc.tile_pool(name="ctxpsum", bufs=2, space="PSUM") as ctxpsum,
            tc.tile_pool(name="mpsum", bufs=1, space="PSUM") as mpsum,
            tc.tile_pool(name="rpsum", bufs=1, space="PSUM") as rpsum,
        ):
            # ---- constants (order matters: wq half, x sq0 halves, wk, wv) ----
            # q/k weights split into kd halves so the first projection rider
            # only waits on a quarter of the weight DMA bytes
            wq_sb = [consts.tile([128, 4, DL], BF16, name=f"wq{i}")
                     for i in range(2)]
            wk_sb = [consts.tile([128, 4, DL], BF16, name=f"wk{i}")
                     for i in range(2)]
            wv_sb = consts.tile([128, KD, DL], BF16)
            wo_sb = consts.tile([128, 2, D], BF16)
            tri_sb = consts.tile([128, 256], BF16)
            onescol_sb = consts.tile([128, 32], BF16)
            selb_sb = consts.tile([33, 128], BF16)
            zb = consts.tile([128, 1], F32)

            def wqt(kd):
                return wq_sb[kd // 4][:, kd % 4, :]

            def wkt(kd):
                return wk_sb[kd // 4][:, kd % 4, :]

            wq_r = wq_d.rearrange("(kd p) j -> p kd j", p=128)
            wk_r = wk_d.rearrange("(kd p) j -> p kd j", p=128)

            # ---- persistent activations ----
            xt = [
                [xpool.tile([128, 512], BF16, name=f"xt_{kd}_{sq}",
                            tag=f"xt_{kd}_{sq}") for sq in range(SQ)]
                for kd in range(KD)
            ]
            qT = [
                [qkpool.tile([128, 512], BF16, name=f"qT_{m2}_{sq}",
                             tag=f"qT_{m2}_{sq}") for sq in range(SQ)]
                for m2 in range(2)
            ]
            kT = [
                [qkpool.tile([128, 512], BF16, name=f"kT_{m2}_{sq}",
                             tag=f"kT_{m2}_{sq}") for sq in range(SQ)]
                for m2 in range(2)
            ]
            vt = [
                vpool.tile([128, NH, DH], BF16, name=f"v_{st}", tag=f"v_{st}")
                for st in range(ST)
            ]
            # ctx tiles split into column halves so the output projection can
            # start as soon as the first normalize half lands
            ctxT = [
                [[cpool.tile([128, 256], BF16, name=f"ctxT_{kc}_{sq}_{hf}",
                             tag=f"ctxT_{kc}_{sq}_{hf}") for hf in range(2)]
                 for sq in range(SQ)]
                for kc in range(2)
            ]

            xT_r = xT_d.rearrange("(kd p) s -> kd p s", p=128)
            # DMA priority: first projection rider (q, kd 0-3) needs wq half
            # 0 + the first four x chunks of sq=0; everything else follows
            nc.sync.dma_start(wq_sb[0][:], wq_r[:, 0:4, :])
            for kd in range(4):
                nc.sync.dma_start(xt[kd][0][:], xT_r[kd, :, 0:512])
            nc.sync.dma_start(wq_sb[1][:], wq_r[:, 4:8, :])
            for kd in range(4, KD):
                nc.sync.dma_start(xt[kd][0][:], xT_r[kd, :, 0:512])
            nc.sync.dma_start(wk_sb[0][:], wk_r[:, 0:4, :])
            nc.sync.dma_start(wk_sb[1][:], wk_r[:, 4:8, :])
            nc.sync.dma_start(tri_sb[:], tri_d[:])
            nc.sync.dma_start(onescol_sb[:], onescol_d[:])
            nc.vector.memset(zb[:], 0.0)
            nc.sync.dma_start(
                wv_sb[:], wv_d.rearrange("(kd p) j -> p kd j", p=128))
            nc.sync.dma_start(selb_sb[:], selb_d[:])
            for sq in range(1, SQ):
                for kd in range(KD):
                    nc.sync.dma_start(
                        xt[kd][sq][:], xT_r[kd, :, sq * 512:(sq + 1) * 512]
                    )
                if sq == 1:
                    nc.sync.dma_start(
                        wo_sb[:], wo_d.rearrange("(kc p) o -> p kc o", p=128))

            # ---- work emitters ----
            # riders alternate between the two single-buffer psum banks so a
            # rider's matmul never waits on the previous rider's evacuation
            _rps = {"i": 0}

            def rider_ps(cols=512):
                pl, tg = ((rpsum, "r"), (mpsum, "m"))[_rps["i"] % 2]
                _rps["i"] += 1
                return pl.tile([128, cols], F32, name="r_ps", tag=tg)

            def emit_qk_group(sq, which, m2):
                wt = wqt if which == "q" else wkt
                dst = qT if which == "q" else kT
                ps = rider_ps()
                for kd in range(KD):
                    nc.tensor.matmul(
                        ps[:],
                        wt(kd)[:, m2 * 128:(m2 + 1) * 128],
                        xt[kd][sq][:],
                        start=(kd == 0),
                        stop=(kd == KD - 1),
                    )
                nc.vector.tensor_copy(dst[m2][sq][:], ps[:])

            def qk_half_riders(sq, which, m2):
                # one q/k projection group split into two riders (4 of the 8
                # contraction chunks each) so a single slot's PE burst stays
                # short; the PSUM accumulator is shared via the closure
                wt = wqt if which == "q" else wkt
                dst = qT if which == "q" else kT
                st = {}

                def h1():
                    ps = rider_ps()
                    for kd in range(4):
                        nc.tensor.matmul(
                            ps[:],
                            wt(kd)[:, m2 * 128:(m2 + 1) * 128],
                            xt[kd][sq][:],
                            start=(kd == 0),
                            stop=False,
                        )
                    st["ps"] = ps

                def h2():
                    ps = st["ps"]
                    for kd in range(4, KD):
                        nc.tensor.matmul(
                            ps[:],
                            wt(kd)[:, m2 * 128:(m2 + 1) * 128],
                            xt[kd][sq][:],
                            start=False,
                            stop=(kd == KD - 1),
                        )
                    nc.vector.tensor_copy(dst[m2][sq][:], ps[:])

                return [h1, h2]

            def emit_v_group(sq, sti):
                st = sq * 4 + sti
                ps = rider_ps(DL)
                for kd in range(KD):
                    nc.tensor.matmul(
                        ps[:],
                        xt[kd][sq][:, sti * 128:(sti + 1) * 128],
                        wv_sb[:, kd, :],
                        start=(kd == 0),
                        stop=(kd == KD - 1),
                    )
                # split the v evacs between ScalarE and DVE (the (j,1)
                # loops these ride are paced by whichever engine is fuller)
                if sti % 2 == 0:
                    nc.scalar.copy(
                        vt[st][:].rearrange("p h d -> p (h d)"), ps[:]
                    )
                else:
                    nc.vector.tensor_copy(
                        vt[st][:].rearrange("p h d -> p (h d)"), ps[:]
                    )

            def emit_wo(j, sti, ot, pool=None, tag=None):
                st = j * 4 + sti
                if pool is None:
                    o_ps = rider_ps()
                else:
                    o_ps = pool.tile([128, 512], F32, name="r_ps", tag=tag)
                for kc in range(2):
                    nc.tensor.matmul(
                        o_ps[:],
                        ctxT[kc][j][sti // 2][:, (sti % 2) * 128:
                                              (sti % 2) * 128 + 128],
                        wo_sb[:, kc, ot * 512:(ot + 1) * 512],
                        start=(kc == 0),
                        stop=(kc == 1),
                    )
                ob = outpool.tile([128, 512], F32, name="ob", tag="ob")
                # alternate the PSUM evac between ScalarE (activation Copy,
                # same act table as Exp) and DVE so neither engine eats the
                # whole 26us; in the LATE loops (these riders run in (j+1,*))
                # ScalarE's exp cadence is the pacer, so keep it clean there
                if (j < 2 or j == SQ - 1) and (sti + ot) % 2 == 0:
                    nc.scalar.copy(ob[:], o_ps[:])
                else:
                    nc.vector.tensor_copy(ob[:], o_ps[:])
                nc.sync.dma_start(
                    out_d[st * 128:(st + 1) * 128, ot * 512:(ot + 1) * 512],
                    ob[:],
                )

            # normalize chain state per (j, p): set at loop end, consumed by
            # riders in the following loop.  q33 holds the per-head exp sums
            # at partitions 0 (head even) and 32 (head odd); selb broadcasts
            # row 0 to out cols [0,64) and row 32 to [64,128), then the
            # reciprocal runs on the already-broadcast (128, 512).
            def make_chain(j, p, q33, c_ps):
                st = {}

                def s_bcast():
                    denb = rider_ps()
                    nc.tensor.matmul(denb[:], selb_sb[:], q33[:],
                                     start=True, stop=True)
                    st["denb"] = denb

                def s_recip():
                    invb = smallpool.tile([128, 512], F32, name="invb",
                                          tag="invb")
                    nc.vector.reciprocal_approx_fast(invb[:], st["denb"][:])
                    st["invb"] = invb
                    if DEBUG and f"invb_{j}_{p}" in dbg:
                        nc.sync.dma_start(dbg[f"invb_{j}_{p}"][:], invb[:])

                def s_mul(hf):
                    nc.vector.tensor_mul(
                        ctxT[p][j][hf][:],
                        c_ps[:, hf * 256:hf * 256 + 256],
                        st["invb"][:, hf * 256:hf * 256 + 256],
                    )

                return [s_bcast, s_recip,
                        lambda: s_mul(0), lambda: s_mul(1)]

            # ---- attention loop for one (j, p) ----
            def attention(j, p, early, bulk, after=[]):
                nkt = 4 * j + 4
                # `early` chain riders run one-per-slot from slot 0 (they
                # recycle the ctx/m PSUM tiles); `bulk` riders (independent of
                # the chain) spread evenly over the loop; `after` riders (the
                # previous q tile's Wo, which READS what the chain writes)
                # must be emitted strictly after the last chain rider.
                E = len(early)
                L = len(bulk)
                A = len(after)
                rem = max(1, nkt - E)
                sched = [
                    ([early[kt]] if kt < E else [])
                    + bulk[(kt * L) // nkt:((kt + 1) * L) // nkt]
                    + (after[((kt - E) * A) // rem:((kt - E + 1) * A) // rem]
                       if kt >= E else [])
                    for kt in range(nkt)
                ]
                c_ps = ctxpsum.tile([128, 512], F32, name="c_ps", tag="ctx")
                exs = {}
                # running bf16 accumulator of all exp tiles (both heads);
                # the PE only sees it once, at loop end
                acc = accpool.tile([128, 1024], BF16, name="acc", tag="acc")
                acc3 = acc[:].rearrange("p (h q) -> p h q", h=2)

                def emit_scores_exp(kt):
                    o = kt - 4 * j
                    q0 = 128 * o if o > 0 else 0
                    s_ps = spsum.tile([128, 1024], F32, name="s_ps", tag="s")
                    for i2 in range(2):
                        hr = i2 * 64
                        nc.tensor.matmul(
                            s_ps[:, i2 * 512 + q0:(i2 + 1) * 512],
                            kT[p][kt // 4][hr:hr + 64,
                                           (kt % 4) * 128:(kt % 4 + 1) * 128],
                            qT[p][j][hr:hr + 64, q0:512],
                            start=True,
                            stop=True,
                        )
                    ex = exppool.tile([128, 1024], BF16, name="ex", tag="ex")
                    if q0 == 0:
                        nc.scalar.activation(
                            ex[:], s_ps[:], AF.Exp, bias=zb[:], scale=SCALE
                        )
                    else:
                        e3 = ex[:].rearrange("p (h q) -> p h q", h=2)
                        s3 = s_ps[:].rearrange("p (h q) -> p h q", h=2)
                        nc.scalar.activation(
                            e3[:, :, q0:512], s3[:, :, q0:512], AF.Exp,
                            bias=zb[:], scale=SCALE
                        )
                    if o >= 0:
                        # triangle mask on the 128-wide diagonal window; on
                        # GpSimd (idle, SBUF-only op) — ctx consumes the
                        # masked tile LAG slots later, so the latency hides
                        e3 = ex[:].rearrange("p (h q) -> p h q", h=2)
                        t3 = tri_sb[:].rearrange("p (h q) -> p h q", h=2)
                        nc.gpsimd.tensor_mul(
                            e3[:, :, q0:q0 + 128], e3[:, :, q0:q0 + 128],
                            t3[:]
                        )
                    exs[kt] = ex
                    if DEBUG and kt == 4 * j + 1 and f"ex_{j}_{p}" in dbg:
                        nc.sync.dma_start(dbg[f"ex_{j}_{p}"][:], ex[:])

                def emit_ctx(kt):
                    o = kt - 4 * j
                    q0 = 128 * o if o > 0 else 0
                    ex = exs[kt]
                    e3 = ex[:].rearrange("p (h q) -> p h q", h=2)
                    for i2 in range(2):
                        nc.tensor.matmul(
                            c_ps[64 * i2:64 * i2 + DH, q0:512],
                            vt[kt][:, 2 * p + i2, :],
                            e3[:, i2, q0:512],
                            start=(kt == 0),
                            stop=(kt == nkt - 1),
                            tile_position=(0, 64 * i2),
                        )
                    if kt % 2 == 1:
                        # fold the (kt-1, kt) exp pair into the running
                        # denominator accumulator on DVE (pair sum, then
                        # in-place accumulate over the pair's valid range).
                        # The LAST pair skips the accumulate: its pair sum
                        # goes to the PE directly as a second denominator
                        # matmul, so the loop-end matmul never waits on the
                        # tail of the DVE add chain.
                        op = kt - 1 - 4 * j
                        qp = 128 * op if op > 0 else 0
                        exprev = exs.pop(kt - 1)
                        ep3 = exprev[:].rearrange("p (h q) -> p h q", h=2)
                        if q0 > qp:
                            # cur tile never wrote [qp, q0); zero it so the
                            # union-range add reads defined data
                            nc.gpsimd.memset(e3[:, :, qp:q0], 0.0)
                        if kt == 1:
                            nc.vector.tensor_add(acc[:], exprev[:], ex[:])
                        else:
                            es = addpool.tile([128, 1024], BF16, name="es",
                                              tag="es")
                            es3 = es[:].rearrange("p (h q) -> p h q", h=2)
                            nc.vector.tensor_add(
                                es3[:, :, qp:512], ep3[:, :, qp:512],
                                e3[:, :, qp:512]
                            )
                            if kt == nkt - 1 and nkt > 4:
                                mstate["last"] = (es3, qp)
                            else:
                                nc.vector.tensor_add(
                                    acc3[:, :, qp:512], acc3[:, :, qp:512],
                                    es3[:, :, qp:512]
                                )

                mstate = {"last": None}
                for kt in range(nkt):
                    # ctx(kt-LAG) is guaranteed-ready (its exp finished slots
                    # ago) — emit it ahead of the scores pair, which may wait
                    # on the s_ps ring, so the PE FIFO head never blocks idle
                    if kt >= LAG:
                        emit_ctx(kt - LAG)
                    emit_scores_exp(kt)
                    for r in sched[kt]:
                        r()
                for kt in range(max(0, nkt - LAG), nkt):
                    emit_ctx(kt)
                exs.clear()

                # single denominator reduction at loop end: one onescol
                # matmul chain per head into 32-row strips (accumulator +
                # the last pair sum), then a (33, 512) evac
                m_ps = mpsum.tile([128, 512], F32, name="m_ps", tag="m")
                for h in range(2):
                    nc.tensor.matmul(
                        m_ps[32 * h:32 * h + 32, :],
                        onescol_sb[:],
                        acc3[:, h, :],
                        start=True,
                        stop=(mstate["last"] is None),
                        tile_position=(0, 32 * h),
                    )
                if mstate["last"] is not None:
                    les3, lqp = mstate["last"]
                    for h in range(2):
                        nc.tensor.matmul(
                            m_ps[32 * h:32 * h + 32, lqp:512],
                            onescol_sb[:],
                            les3[:, h, lqp:512],
                            start=False,
                            stop=True,
                            tile_position=(0, 32 * h),
                        )
                q33 = smallpool.tile([33, 512], BF16, name="q33", tag="q33")
                nc.vector.tensor_copy(q33[:], m_ps[0:33, :])
                if DEBUG and f"q4_{j}_{p}" in dbg:
                    nc.sync.dma_start(dbg[f"q4_{j}_{p}"][0:33, :], q33[:])
                return make_chain(j, p, q33, c_ps)

            # ---- prologue: just enough to start (j=0, p=0); half-riders so
            # the first matmul only waits on wq half 0 + x chunks 0-3 ----
            for r in qk_half_riders(0, "q", 0):
                r()
            for r in qk_half_riders(0, "k", 0):
                r()
            emit_v_group(0, 0)
            emit_v_group(0, 1)

            def qkv_riders(sq):
                return [
                    lambda s=sq: emit_qk_group(s, "q", 0),
                    lambda s=sq: emit_qk_group(s, "k", 0),
                    lambda s=sq: emit_qk_group(s, "q", 1),
                    lambda s=sq: emit_qk_group(s, "k", 1),
                    lambda s=sq: emit_v_group(s, 0),
                    lambda s=sq: emit_v_group(s, 1),
                    lambda s=sq: emit_v_group(s, 2),
                    lambda s=sq: emit_v_group(s, 3),
                ]

            def wo_riders(j):
                return [
                    (lambda jj=j, s=sti, o=ot: emit_wo(jj, s, o))
                    for sti in range(4) for ot in range(2)
                ]

            # ---- main loops ----
            chain = {}
            chain[(0, 0)] = attention(
                0, 0, [],
                [lambda: emit_qk_group(0, "q", 1),
                 lambda: emit_qk_group(0, "k", 1),
                 lambda: emit_v_group(0, 2),
                 lambda: emit_v_group(0, 3)],
            )
            chain[(0, 1)] = attention(0, 1, chain[(0, 0)], qkv_riders(1))
            # sq=3's k/v projections ride j=3's own (ScalarE-bound) loops —
            # they are only consumed from kt=12 — so PE-bound j=2 stays lean
            for j in range(1, SQ):
                # q/k of the next q tile ride (j,0); its v groups ride (j,1).
                # wo(j-1) riders split 4+4 across (j,0) and (j,1) so their
                # PSUM evacuations don't pile onto one loop's Scalar/DVE
                # budget (the exp cadence paces the late loops)
                qkv = qkv_riders(j + 1) if j + 1 < SQ else []
                wos = wo_riders(j - 1)
                chain[(j, 0)] = attention(j, 0, chain[(j - 1, 1)], qkv[:4],
                                          after=wos[:4])
                chain[(j, 1)] = attention(j, 1, chain[(j, 0)],
                                          qkv[4:] + wos[4:])

            # ---- epilogue ----
            for step in chain[(SQ - 1, 1)]:
                step()
            # rotate the last Wo tiles across the (now idle) PSUM rings so
            # they pipeline instead of serializing on the single rider bank
            epools = [(rpsum, "r"), (mpsum, "m"), (spsum, "s")]
            ei = 0
            for sti in range(4):
                for ot in range(2):
                    p, t = epools[ei % 3]
                    ei += 1
                    emit_wo(SQ - 1, sti, ot, pool=p, tag=t)

    nc.compile()
    return nc


def _get_nc():
    global _NC
    if _NC is None:
        _NC = _build_nc()
    return _NC


def _bf16(a):
    return np.ascontiguousarray(a).astype(ml_dtypes.bfloat16)


def kernel(x, Wq, Wk, Wv, Wo, bo):
    global LAST_RESULTS
    x = np.asarray(x, dtype=np.float32)
    Wq = np.asarray(Wq, dtype=np.float32)
    Wk = np.asarray(Wk, dtype=np.float32)
    Wv = np.asarray(Wv, dtype=np.float32)
    Wo = np.asarray(Wo, dtype=np.float32)
    bo = np.asarray(bo, dtype=np.float32)

    xT = [_bf16(x[b].T) for b in range(B)]          # (D, S)
    WqT = np.ascontiguousarray(Wq.T)                # (D, D): col slice = head rows
    WkT = np.ascontiguousarray(Wk.T)
    WvT = np.ascontiguousarray(Wv.T)
    WoT = np.ascontiguousarray(Wo.T)                # (D, D): row slice = ctx dims

    kk = np.arange(128)[:, None]
    cc = np.arange(128)[None, :]
    tri1 = (cc >= kk).astype(np.float32)            # (128, 128)
    tri = np.concatenate([tri1, tri1], axis=1).astype(ml_dtypes.bfloat16)

    onescol = np.zeros((128, 32), dtype=np.float32)
    onescol[:, 0] = 1.0
    onescol = onescol.astype(ml_dtypes.bfloat16)
    selb = np.zeros((33, 128), dtype=np.float32)
    selb[0, 0:64] = 1.0                             # head-even denom row
    selb[32, 64:128] = 1.0                          # head-odd denom row
    selb = selb.astype(ml_dtypes.bfloat16)

    in_maps = []
    for c in range(NCORES):
        b, g = divmod(c, 4)
        sl = slice(g * DL, (g + 1) * DL)
        in_maps.append(
            {
                "xT": xT[b],
                "wqT": _bf16(WqT[:, sl]),
                "wkT": _bf16(WkT[:, sl]),
                "wvT": _bf16(WvT[:, sl]),
                "woT": _bf16(WoT[sl, :]),
                "tri": tri,
                "onescol": onescol,
                "selb": selb,
            }
        )

    nc = _get_nc()
    results = run_bass_kernel_spmd(
        nc, in_maps, core_ids=list(range(NCORES)), trace=TRACE
    )
    LAST_RESULTS = results

    out = np.zeros((B, S, D), dtype=np.float32)
    for c in range(NCORES):
        out[c // 4] += results.results[c]["out"]
    out += bo[None, None, :]
    return out



# revision 36
# speedup vs baseline: 1.0515x; 1.0515x over previous
"""Trainium2 Bass kernel for causal multi-head attention.

Problem: B=2, S=2048, D=1024, H=16 heads (DH=64), causal, fp32 reference.

Sharding over 8 NeuronCores: core c handles batch b = c//4 and head group
g = c%4 (4 heads each).  Wq/Wk/Wv are split column-wise (by output head),
Wo row-wise; per-core partial outputs are summed on the host (row-parallel
Wo => partial sums), then bo is added.

Per-core device kernel (matmul operands bf16, fp32 PSUM accumulation):
  qT/kT = W @ xT            (64, 2048) per head, head pairs stacked on 128 parts
  v     = x @ WvT           (2048, 256)
  S^T   = k q^T             scores transposed: (s_k, s_q) tiles; diagonal
                            blocks shrunk to the causally-valid q range
  P     = exp(0.125 * S^T)  ScalarE (only exp runs there); the 128-wide
                            triangle at the diagonal masked via DVE mul
  ctx^T = v^T P             accumulated over k tiles in PSUM
  sums: DVE pre-adds each (kt-1, kt) exp-tile pair, then one onescol
        matmul per head per PAIR accumulates into two 32-row strips
        (rows 0 / 32 of m_ps), halving the denominator's PE cost
  denominators: single (33, 512) bf16 evac -> selb broadcast matmul ->
                DVE reciprocal on the broadcast (128, 512) -> DVE
                normalize multiply, riding the next q tile's k loop
  out   = ctx @ WoT         (2048, 1024) fp32 partial, riding the next
                            loop; PSUM->SBUF evac on the idle GpSimd

All non-attention work (projections of the next q tile, output projection
of the previous one, the normalize chain) is queued as "riders" that are
drained at a fixed cadence inside the attention k loops so PE/DVE/ACT all
stay busy; ScalarE's exp paces the steady state.
"""

import sys
import types

sys.path.insert(0, "/opt/trn_rl_repo")

import numpy as np
import ml_dtypes

import concourse.bass as bass
import concourse.bacc as bacc
import concourse.mybir as mybir
import concourse.tile as tile
from concourse.bass_utils import run_bass_kernel_spmd

# Some images lack antenv.axon_hooks, which bass_utils imports when
# trace=True (or BASS_TRACE is set).  Recreate it with the ctypes-based
# NTFF hook from trn_boot when possible; degrade to a None hook (trace
# skipped, run still works) otherwise.
try:
    import antenv.axon_hooks  # noqa: F401
except ImportError:
    try:
        import antenv

        _hook = None
        try:
            from trn_agent_boot.trn_boot import _ntff_profile_via_ctypes

            _hook = _ntff_profile_via_ctypes("/opt/axon/libaxon_pjrt.so")
        except Exception:  # noqa: BLE001
            pass
        _mod = types.ModuleType("antenv.axon_hooks")
        _mod._hook = _hook
        _mod.set_axon_ntff_profile_hook = lambda h: setattr(_mod, "_hook", h)
        _mod.get_axon_ntff_profile_hook = lambda: _mod._hook
        sys.modules["antenv.axon_hooks"] = _mod
        antenv.axon_hooks = _mod
    except Exception:  # noqa: BLE001
        pass

BF16 = mybir.dt.bfloat16
F32 = mybir.dt.float32
AF = mybir.ActivationFunctionType

B, S, D, H = 2, 2048, 1024, 16
DH = D // H            # 64
NCORES = 8
NH = 4                 # heads per core
DL = NH * DH           # 256 local head dims per core
KD = D // 128          # 8 contraction chunks for projections
SQ = S // 512          # 4 q tiles of 512
ST = S // 128          # 16 s tiles of 128
SCALE = DH ** -0.5     # 0.125
LAG = 3                # ctx trails scores by LAG k-tiles

_NC = None

TRACE = False
LAST_RESULTS = None
DEBUG = False


def _build_nc():
    nc = bacc.Bacc("TRN2", target_bir_lowering=False, debug=False,
                   num_devices=NCORES)

    xT_d = nc.dram_tensor("xT", [D, S], BF16, kind="ExternalInput")
    wq_d = nc.dram_tensor("wqT", [D, DL], BF16, kind="ExternalInput")
    wk_d = nc.dram_tensor("wkT", [D, DL], BF16, kind="ExternalInput")
    wv_d = nc.dram_tensor("wvT", [D, DL], BF16, kind="ExternalInput")
    wo_d = nc.dram_tensor("woT", [DL, D], BF16, kind="ExternalInput")
    # [128, 2, 128] lower-triangle (q >= k) mask, duplicated per head half
    tri_d = nc.dram_tensor("tri", [128, 256], BF16, kind="ExternalInput")
    # ones in column 0, zeros elsewhere (M=32 so strip rows are zero-filled)
    onescol_d = nc.dram_tensor("onescol", [128, 32], BF16, kind="ExternalInput")
    # denominator broadcast selector: row 0 -> ones at cols [0,64),
    # row 32 -> ones at cols [64,128); all other rows zero
    selb_d = nc.dram_tensor("selb", [33, 128], BF16, kind="ExternalInput")
    out_d = nc.dram_tensor("out", [S, D], F32, kind="ExternalOutput")
    dbg = {}
    if DEBUG:
        for j in (1, 2):
            for p in (0, 1):
                dbg[f"q4_{j}_{p}"] = nc.dram_tensor(
                    f"dbg_q4_{j}_{p}", [128, 512], F32, kind="ExternalOutput")
                dbg[f"inv2_{j}_{p}"] = nc.dram_tensor(
                    f"dbg_inv2_{j}_{p}", [2, 512], F32, kind="ExternalOutput")
                dbg[f"invb_{j}_{p}"] = nc.dram_tensor(
                    f"dbg_invb_{j}_{p}", [128, 512], F32, kind="ExternalOutput")
                dbg[f"ctxT_{j}_{p}"] = nc.dram_tensor(
                    f"dbg_ctxT_{j}_{p}", [128, 512], BF16, kind="ExternalOutput")
                dbg[f"ex_{j}_{p}"] = nc.dram_tensor(
                    f"dbg_ex_{j}_{p}", [128, 1024], BF16, kind="ExternalOutput")

    with tile.TileContext(nc) as tc:
        with (
            tc.tile_pool(name="consts", bufs=1) as consts,
            tc.tile_pool(name="xpool", bufs=1) as xpool,
            tc.tile_pool(name="qkpool", bufs=1) as qkpool,
            tc.tile_pool(name="vpool", bufs=1) as vpool,
            tc.tile_pool(name="cpool", bufs=1) as cpool,
            tc.tile_pool(name="exppool", bufs=8) as exppool,
            tc.tile_pool(name="addpool", bufs=3) as addpool,
            tc.tile_pool(name="accpool", bufs=2) as accpool,
            tc.tile_pool(name="smallpool", bufs=2) as smallpool,
            tc.tile_pool(name="outpool", bufs=6) as outpool,
            tc.tile_pool(name="spsum", bufs=2, space="PSUM") as spsum,
            tc.tile_pool(name="ctxpsum", bufs=2, space="PSUM") as ctxpsum,
            tc.tile_pool(name="mpsum", bufs=1, space="PSUM") as mpsum,
            tc.tile_pool(name="rpsum", bufs=1, space="PSUM") as rpsum,
        ):
            # ---- constants (order matters: wq half, x sq0 halves, wk, wv) ----
            # q/k weights split into kd halves so the first projection rider
            # only waits on a quarter of the weight DMA bytes
            wq_sb = [consts.tile([128, 4, DL], BF16, name=f"wq{i}")
                     for i in range(2)]
            wk_sb = [consts.tile([128, 4, DL], BF16, name=f"wk{i}")
                     for i in range(2)]
            wv_sb = consts.tile([128, KD, DL], BF16)
            wo_sb = consts.tile([128, 2, D], BF16)
            tri_sb = consts.tile([128, 256], BF16)
            onescol_sb = consts.tile([128, 32], BF16)
            selb_sb = consts.tile([33, 128], BF16)
            zb = consts.tile([128, 1], F32)

            def wqt(kd):
                return wq_sb[kd // 4][:, kd % 4, :]

            def wkt(kd):
                return wk_sb[kd // 4][:, kd % 4, :]

            wq_r = wq_d.rearrange("(kd p) j -> p kd j", p=128)
            wk_r = wk_d.rearrange("(kd p) j -> p kd j", p=128)

            # ---- persistent activations ----
            xt = [
                [xpool.tile([128, 512], BF16, name=f"xt_{kd}_{sq}",
                            tag=f"xt_{kd}_{sq}") for sq in range(SQ)]
                for kd in range(KD)
            ]
            qT = [
                [qkpool.tile([128, 512], BF16, name=f"qT_{m2}_{sq}",
                             tag=f"qT_{m2}_{sq}") for sq in range(SQ)]
                for m2 in range(2)
            ]
            kT = [
                [qkpool.tile([128, 512], BF16, name=f"kT_{m2}_{sq}",
                             tag=f"kT_{m2}_{sq}") for sq in range(SQ)]
                for m2 in range(2)
            ]
            vt = [
                vpool.tile([128, NH, DH], BF16, name=f"v_{st}", tag=f"v_{st}")
                for st in range(ST)
            ]
            # ctx tiles split into column halves so the output projection can
            # start as soon as the first normalize half lands
            ctxT = [
                [[cpool.tile([128, 256], BF16, name=f"ctxT_{kc}_{sq}_{hf}",
                             tag=f"ctxT_{kc}_{sq}_{hf}") for hf in range(2)]
                 for sq in range(SQ)]
                for kc in range(2)
            ]

            xT_r = xT_d.rearrange("(kd p) s -> kd p s", p=128)
            # DMA priority: first projection rider (q, kd 0-3) needs wq half
            # 0 + the first four x chunks of sq=0; everything else follows
            nc.sync.dma_start(wq_sb[0][:], wq_r[:, 0:4, :])
            for kd in range(4):
                nc.sync.dma_start(xt[kd][0][:], xT_r[kd, :, 0:512])
            nc.sync.dma_start(wq_sb[1][:], wq_r[:, 4:8, :])
            for kd in range(4, KD):
                nc.sync.dma_start(xt[kd][0][:], xT_r[kd, :, 0:512])
            nc.sync.dma_start(wk_sb[0][:], wk_r[:, 0:4, :])
            nc.sync.dma_start(wk_sb[1][:], wk_r[:, 4:8, :])
            nc.sync.dma_start(tri_sb[:], tri_d[:])
            nc.sync.dma_start(onescol_sb[:], onescol_d[:])
            nc.vector.memset(zb[:], 0.0)
            nc.sync.dma_start(
                wv_sb[:], wv_d.rearrange("(kd p) j -> p kd j", p=128))
            nc.sync.dma_start(selb_sb[:], selb_d[:])
            for sq in range(1, SQ):
                for kd in range(KD):
                    nc.sync.dma_start(
                        xt[kd][sq][:], xT_r[kd, :, sq * 512:(sq + 1) * 512]
                    )
                if sq == 1:
                    nc.sync.dma_start(
                        wo_sb[:], wo_d.rearrange("(kc p) o -> p kc o", p=128))

            # ---- work emitters ----
            # riders alternate between the two single-buffer psum banks so a
            # rider's matmul never waits on the previous rider's evacuation
            _rps = {"i": 0}

            def rider_ps(cols=512):
                pl, tg = ((rpsum, "r"), (mpsum, "m"))[_rps["i"] % 2]
                _rps["i"] += 1
                return pl.tile([128, cols], F32, name="r_ps", tag=tg)

            def emit_qk_group(sq, which, m2):
                wt = wqt if which == "q" else wkt
                dst = qT if which == "q" else kT
                ps = rider_ps()
                for kd in range(KD):
                    nc.tensor.matmul(
                        ps[:],
                        wt(kd)[:, m2 * 128:(m2 + 1) * 128],
                        xt[kd][sq][:],
                        start=(kd == 0),
                        stop=(kd == KD - 1),
                    )
                nc.vector.tensor_copy(dst[m2][sq][:], ps[:])

            def qk_half_riders(sq, which, m2):
                # one q/k projection group split into two riders (4 of the 8
                # contraction chunks each) so a single slot's PE burst stays
                # short; the PSUM accumulator is shared via the closure
                wt = wqt if which == "q" else wkt
                dst = qT if which == "q" else kT
                st = {}

                def h1():
                    ps = rider_ps()
                    for kd in range(4):
                        nc.tensor.matmul(
                            ps[:],
                            wt(kd)[:, m2 * 128:(m2 + 1) * 128],
                            xt[kd][sq][:],
                            start=(kd == 0),
                            stop=False,
                        )
                    st["ps"] = ps

                def h2():
                    ps = st["ps"]
                    for kd in range(4, KD):
                        nc.tensor.matmul(
                            ps[:],
                            wt(kd)[:, m2 * 128:(m2 + 1) * 128],
                            xt[kd][sq][:],
                            start=False,
                            stop=(kd == KD - 1),
                        )
                    nc.vector.tensor_copy(dst[m2][sq][:], ps[:])

                return [h1, h2]

            def emit_v_group(sq, sti):
                st = sq * 4 + sti
                ps = rider_ps(DL)
                for kd in range(KD):
                    nc.tensor.matmul(
                        ps[:],
                        xt[kd][sq][:, sti * 128:(sti + 1) * 128],
                        wv_sb[:, kd, :],
                        start=(kd == 0),
                        stop=(kd == KD - 1),
                    )
                # split the v evacs between ScalarE and DVE (the (j,1)
                # loops these ride are paced by whichever engine is fuller)
                if sti % 2 == 0:
                    nc.scalar.copy(
                        vt[st][:].rearrange("p h d -> p (h d)"), ps[:]
                    )
                else:
                    nc.vector.tensor_copy(
                        vt[st][:].rearrange("p h d -> p (h d)"), ps[:]
                    )

            def emit_wo(j, sti, ot, pool=None, tag=None):
                st = j * 4 + sti
                if pool is None:
                    o_ps = rider_ps()
                else:
                    o_ps = pool.tile([128, 512], F32, name="r_ps", tag=tag)
                for kc in range(2):
                    nc.tensor.matmul(
                        o_ps[:],
                        ctxT[kc][j][sti // 2][:, (sti % 2) * 128:
                                              (sti % 2) * 128 + 128],
                        wo_sb[:, kc, ot * 512:(ot + 1) * 512],
                        start=(kc == 0),
                        stop=(kc == 1),
                    )
                ob = outpool.tile([128, 512], F32, name="ob", tag="ob")
                # alternate the PSUM evac between ScalarE (activation Copy,
                # same act table as Exp) and DVE so neither engine eats the
                # whole 26us; in the LATE loops (these riders run in (j+1,*))
                # ScalarE's exp cadence is the pacer, so keep it clean there
                if (j < 2 or j == SQ - 1) and (sti + ot) % 2 == 0:
                    nc.scalar.copy(ob[:], o_ps[:])
                else:
                    nc.vector.tensor_copy(ob[:], o_ps[:])
                nc.sync.dma_start(
                    out_d[st * 128:(st + 1) * 128, ot * 512:(ot + 1) * 512],
                    ob[:],
                )

            # normalize chain state per (j, p): set at loop end, consumed by
            # riders in the following loop.  q33 holds the per-head exp sums
            # at partitions 0 (head even) and 32 (head odd); selb broadcasts
            # row 0 to out cols [0,64) and row 32 to [64,128), then the
            # reciprocal runs on the already-broadcast (128, 512).
            def make_chain(j, p, q33, c_ps):
                st = {}

                def s_bcast():
                    denb = rider_ps()
                    nc.tensor.matmul(denb[:], selb_sb[:], q33[:],
                                     start=True, stop=True)
                    st["denb"] = denb

                def s_recip():
                    invb = smallpool.tile([128, 512], F32, name="invb",
                                          tag="invb")
                    nc.vector.reciprocal_approx_fast(invb[:], st["denb"][:])
                    st["invb"] = invb
                    if DEBUG and f"invb_{j}_{p}" in dbg:
                        nc.sync.dma_start(dbg[f"invb_{j}_{p}"][:], invb[:])

                def s_mul(hf):
                    nc.vector.tensor_mul(
                        ctxT[p][j][hf][:],
                        c_ps[:, hf * 256:hf * 256 + 256],
                        st["invb"][:, hf * 256:hf * 256 + 256],
                    )

                return [s_bcast, s_recip,
                        lambda: s_mul(0), lambda: s_mul(1)]

            # ---- attention loop for one (j, p) ----
            def attention(j, p, early, bulk, after=[]):
                nkt = 4 * j + 4
                # `early` chain riders run one-per-slot from slot 0 (they
                # recycle the ctx/m PSUM tiles); `bulk` riders (independent of
                # the chain) spread evenly over the loop; `after` riders (the
                # previous q tile's Wo, which READS what the chain writes)
                # must be emitted strictly after the last chain rider.
                E = len(early)
                L = len(bulk)
                A = len(after)
                rem = max(1, nkt - E)
                sched = [
                    ([early[kt]] if kt < E else [])
                    + bulk[(kt * L) // nkt:((kt + 1) * L) // nkt]
                    + (after[((kt - E) * A) // rem:((kt - E + 1) * A) // rem]
                       if kt >= E else [])
                    for kt in range(nkt)
                ]
                c_ps = ctxpsum.tile([128, 512], F32, name="c_ps", tag="ctx")
                exs = {}
                # running bf16 accumulator of all exp tiles (both heads);
                # the PE only sees it once, at loop end
                acc = accpool.tile([128, 1024], BF16, name="acc", tag="acc")
                acc3 = acc[:].rearrange("p (h q) -> p h q", h=2)

                def emit_scores_exp(kt):
                    o = kt - 4 * j
                    q0 = 128 * o if o > 0 else 0
                    s_ps = spsum.tile([128, 1024], F32, name="s_ps", tag="s")
                    for i2 in range(2):
                        hr = i2 * 64
                        nc.tensor.matmul(
                            s_ps[:, i2 * 512 + q0:(i2 + 1) * 512],
                            kT[p][kt // 4][hr:hr + 64,
                                           (kt % 4) * 128:(kt % 4 + 1) * 128],
                            qT[p][j][hr:hr + 64, q0:512],
                            start=True,
                            stop=True,
                        )
                    ex = exppool.tile([128, 1024], BF16, name="ex", tag="ex")
                    if q0 == 0:
                        nc.scalar.activation(
                            ex[:], s_ps[:], AF.Exp, bias=zb[:], scale=SCALE
                        )
                    else:
                        e3 = ex[:].rearrange("p (h q) -> p h q", h=2)
                        s3 = s_ps[:].rearrange("p (h q) -> p h q", h=2)
                        nc.scalar.activation(
                            e3[:, :, q0:512], s3[:, :, q0:512], AF.Exp,
                            bias=zb[:], scale=SCALE
                        )
                    if o >= 0:
                        # triangle mask on the 128-wide diagonal window
                        e3 = ex[:].rearrange("p (h q) -> p h q", h=2)
                        t3 = tri_sb[:].rearrange("p (h q) -> p h q", h=2)
                        nc.vector.tensor_mul(
                            e3[:, :, q0:q0 + 128], e3[:, :, q0:q0 + 128],
                            t3[:]
                        )
                    exs[kt] = ex
                    if DEBUG and kt == 4 * j + 1 and f"ex_{j}_{p}" in dbg:
                        nc.sync.dma_start(dbg[f"ex_{j}_{p}"][:], ex[:])

                def emit_ctx(kt):
                    o = kt - 4 * j
                    q0 = 128 * o if o > 0 else 0
                    ex = exs[kt]
                    e3 = ex[:].rearrange("p (h q) -> p h q", h=2)
                    for i2 in range(2):
                        nc.tensor.matmul(
                            c_ps[64 * i2:64 * i2 + DH, q0:512],
                            vt[kt][:, 2 * p + i2, :],
                            e3[:, i2, q0:512],
                            start=(kt == 0),
                            stop=(kt == nkt - 1),
                            tile_position=(0, 64 * i2),
                        )
                    if kt % 2 == 1:
                        # fold the (kt-1, kt) exp pair into the running
                        # denominator accumulator on DVE (pair sum, then
                        # in-place accumulate over the pair's valid range).
                        # The LAST pair skips the accumulate: its pair sum
                        # goes to the PE directly as a second denominator
                        # matmul, so the loop-end matmul never waits on the
                        # tail of the DVE add chain.
                        op = kt - 1 - 4 * j
                        qp = 128 * op if op > 0 else 0
                        exprev = exs.pop(kt - 1)
                        ep3 = exprev[:].rearrange("p (h q) -> p h q", h=2)
                        if q0 > qp:
                            # cur tile never wrote [qp, q0); zero it so the
                            # union-range add reads defined data
                            nc.gpsimd.memset(e3[:, :, qp:q0], 0.0)
                        if kt == 1:
                            nc.vector.tensor_add(acc[:], exprev[:], ex[:])
                        else:
                            es = addpool.tile([128, 1024], BF16, name="es",
                                              tag="es")
                            es3 = es[:].rearrange("p (h q) -> p h q", h=2)
                            nc.vector.tensor_add(
                                es3[:, :, qp:512], ep3[:, :, qp:512],
                                e3[:, :, qp:512]
                            )
                            if kt == nkt - 1 and nkt > 4:
                                mstate["last"] = (es3, qp)
                            else:
                                nc.vector.tensor_add(
                                    acc3[:, :, qp:512], acc3[:, :, qp:512],
                                    es3[:, :, qp:512]
                                )

                mstate = {"last": None}
                for kt in range(nkt):
                    # ctx(kt-LAG) is guaranteed-ready (its exp finished slots
                    # ago) — emit it ahead of the scores pair, which may wait
                    # on the s_ps ring, so the PE FIFO head never blocks idle
                    if kt >= LAG:
                        emit_ctx(kt - LAG)
                    emit_scores_exp(kt)
                    for r in sched[kt]:
                        r()
                for kt in range(max(0, nkt - LAG), nkt):
                    emit_ctx(kt)
                exs.clear()

                # single denominator reduction at loop end: one onescol
                # matmul chain per head into 32-row strips (accumulator +
                # the last pair sum), then a (33, 512) evac
                m_ps = mpsum.tile([128, 512], F32, name="m_ps", tag="m")
                for h in range(2):
                    nc.tensor.matmul(
                        m_ps[32 * h:32 * h + 32, :],
                        onescol_sb[:],
                        acc3[:, h, :],
                        start=True,
                        stop=(mstate["last"] is None),
                        tile_position=(0, 32 * h),
                    )
                if mstate["last"] is not None:
                    les3, lqp = mstate["last"]
                    for h in range(2):
                        nc.tensor.matmul(
                            m_ps[32 * h:32 * h + 32, lqp:512],
                            onescol_sb[:],
                            les3[:, h, lqp:512],
                            start=False,
                            stop=True,
                            tile_position=(0, 32 * h),
                        )
                q33 = smallpool.tile([33, 512], BF16, name="q33", tag="q33")
                if (j, p) == (SQ - 1, 1):
                    # last loop: DVE still drains the tail adds; ScalarE is
                    # free once the final exp lands, so the epilogue chain
                    # (bcast -> recip -> muls -> wo) starts sooner
                    nc.scalar.copy(q33[:], m_ps[0:33, :])
                else:
                    nc.vector.tensor_copy(q33[:], m_ps[0:33, :])
                if DEBUG and f"q4_{j}_{p}" in dbg:
                    nc.sync.dma_start(dbg[f"q4_{j}_{p}"][0:33, :], q33[:])
                return make_chain(j, p, q33, c_ps)

            # ---- prologue: just enough to start (j=0, p=0); half-riders so
            # the first matmul only waits on wq half 0 + x chunks 0-3 ----
            for r in qk_half_riders(0, "q", 0):
                r()
            for r in qk_half_riders(0, "k", 0):
                r()
            emit_v_group(0, 0)
            emit_v_group(0, 1)

            def qkv_riders(sq):
                return [
                    lambda s=sq: emit_qk_group(s, "q", 0),
                    lambda s=sq: emit_qk_group(s, "k", 0),
                    lambda s=sq: emit_qk_group(s, "q", 1),
                    lambda s=sq: emit_qk_group(s, "k", 1),
                    lambda s=sq: emit_v_group(s, 0),
                    lambda s=sq: emit_v_group(s, 1),
                    lambda s=sq: emit_v_group(s, 2),
                    lambda s=sq: emit_v_group(s, 3),
                ]

            def wo_riders(j):
                return [
                    (lambda jj=j, s=sti, o=ot: emit_wo(jj, s, o))
                    for sti in range(4) for ot in range(2)
                ]

            # ---- main loops ----
            chain = {}
            chain[(0, 0)] = attention(
                0, 0, [],
                [lambda: emit_qk_group(0, "q", 1),
                 lambda: emit_qk_group(0, "k", 1),
                 lambda: emit_v_group(0, 2),
                 lambda: emit_v_group(0, 3)],
            )
            chain[(0, 1)] = attention(0, 1, chain[(0, 0)], qkv_riders(1))
            # sq=3's k/v projections ride j=3's own (ScalarE-bound) loops —
            # they are only consumed from kt=12 — so PE-bound j=2 stays lean
            for j in range(1, SQ):
                # q/k of the next q tile ride (j,0); its v groups ride (j,1).
                # wo(j-1) riders split 4+4 across (j,0) and (j,1) so their
                # PSUM evacuations don't pile onto one loop's Scalar/DVE
                # budget (the exp cadence paces the late loops)
                qkv = qkv_riders(j + 1) if j + 1 < SQ else []
                wos = wo_riders(j - 1)
                chain[(j, 0)] = attention(j, 0, chain[(j - 1, 1)], qkv[:4],
                                          after=wos[:4])
                chain[(j, 1)] = attention(j, 1, chain[(j, 0)],
                                          qkv[4:] + wos[4:])

            # ---- epilogue ----
            for step in chain[(SQ - 1, 1)]:
                step()
            # rotate the last Wo tiles across the (now idle) PSUM rings so
            # they pipeline instead of serializing on the single rider bank
            epools = [(rpsum, "r"), (mpsum, "m"), (spsum, "s")]
            ei = 0
            for sti in range(4):
                for ot in range(2):
                    p, t = epools[ei % 3]
                    ei += 1
                    emit_wo(SQ - 1, sti, ot, pool=p, tag=t)

    nc.compile()
    return nc


def _get_nc():
    global _NC
    if _NC is None:
        _NC = _build_nc()
    return _NC


def _bf16(a):
    return np.ascontiguousarray(a).astype(ml_dtypes.bfloat16)


def kernel(x, Wq, Wk, Wv, Wo, bo):
    global LAST_RESULTS
    x = np.asarray(x, dtype=np.float32)
    Wq = np.asarray(Wq, dtype=np.float32)
    Wk = np.asarray(Wk, dtype=np.float32)
    Wv = np.asarray(Wv, dtype=np.float32)
    Wo = np.asarray(Wo, dtype=np.float32)
    bo = np.asarray(bo, dtype=np.float32)

    xT = [_bf16(x[b].T) for b in range(B)]          # (D, S)
    WqT = np.ascontiguousarray(Wq.T)                # (D, D): col slice = head rows
    WkT = np.ascontiguousarray(Wk.T)
    WvT = np.ascontiguousarray(Wv.T)
    WoT = np.ascontiguousarray(Wo.T)                # (D, D): row slice = ctx dims

    kk = np.arange(128)[:, None]
    cc = np.arange(128)[None, :]
    tri1 = (cc >= kk).astype(np.float32)            # (128, 128)
    tri = np.concatenate([tri1, tri1], axis=1).astype(ml_dtypes.bfloat16)

    onescol = np.zeros((128, 32), dtype=np.float32)
    onescol[:, 0] = 1.0
    onescol = onescol.astype(ml_dtypes.bfloat16)
    selb = np.zeros((33, 128), dtype=np.float32)
    selb[0, 0:64] = 1.0                             # head-even denom row
    selb[32, 64:128] = 1.0                          # head-odd denom row
    selb = selb.astype(ml_dtypes.bfloat16)

    in_maps = []
    for c in range(NCORES):
        b, g = divmod(c, 4)
        sl = slice(g * DL, (g + 1) * DL)
        in_maps.append(
            {
                "xT": xT[b],
                "wqT": _bf16(WqT[:, sl]),
                "wkT": _bf16(WkT[:, sl]),
                "wvT": _bf16(WvT[:, sl]),
                "woT": _bf16(WoT[sl, :]),
                "tri": tri,
                "onescol": onescol,
                "selb": selb,
            }
        )

    nc = _get_nc()
    results = run_bass_kernel_spmd(
        nc, in_maps, core_ids=list(range(NCORES)), trace=TRACE
    )
    LAST_RESULTS = results

    out = np.zeros((B, S, D), dtype=np.float32)
    for c in range(NCORES):
        out[c // 4] += results.results[c]["out"]
    out += bo[None, None, :]
    return out



# revision 37
# speedup vs baseline: 1.0617x; 1.0097x over previous
"""Trainium2 Bass kernel for causal multi-head attention.

Problem: B=2, S=2048, D=1024, H=16 heads (DH=64), causal, fp32 reference.

Sharding over 8 NeuronCores: core c handles batch b = c//4 and head group
g = c%4 (4 heads each).  Wq/Wk/Wv are split column-wise (by output head),
Wo row-wise; per-core partial outputs are summed on the host (row-parallel
Wo => partial sums), then bo is added.

Per-core device kernel (matmul operands bf16, fp32 PSUM accumulation):
  qT/kT = W @ xT            (64, 2048) per head, head pairs stacked on 128 parts
  v     = x @ WvT           (2048, 256)
  S^T   = k q^T             scores transposed: (s_k, s_q) tiles; diagonal
                            blocks shrunk to the causally-valid q range
  P     = exp(0.125 * S^T)  ScalarE (only exp runs there); the 128-wide
                            triangle at the diagonal masked via DVE mul
  ctx^T = v^T P             accumulated over k tiles in PSUM
  denominators: DVE keeps a running bf16 accumulator of exp-tile PAIR
        sums (the last pair goes to the PE directly), so the PE sees
        the whole reduction as two onescol matmuls per loop into 32-row
        strips; then a (33, 512) evac -> selb broadcast matmul -> DVE
        reciprocal on the broadcast (128, 512) -> DVE normalize
        multiply (column halves), riding the next q tile's k loop
  out   = ctx @ WoT         (2048, 1024) fp32 partial, riders split 4+4
                            across the two following loops; PSUM evac
                            alternates ScalarE / DVE by loop phase

All non-attention work (projections of the next q tile, output projection
of the previous one, the normalize chain) is queued as "riders" that are
drained at a fixed cadence inside the attention k loops so PE/DVE/ACT all
stay busy; riders alternate between two single-buffer PSUM banks so they
never serialize on each other's evacuation.  ScalarE's exp paces the late
loops; the PE (at its ramped 2.4 GHz clock) paces the rest.
"""

import sys
import types

sys.path.insert(0, "/opt/trn_rl_repo")

import numpy as np
import ml_dtypes

import concourse.bass as bass
import concourse.bacc as bacc
import concourse.mybir as mybir
import concourse.tile as tile
from concourse.bass_utils import run_bass_kernel_spmd

# Some images lack antenv.axon_hooks, which bass_utils imports when
# trace=True (or BASS_TRACE is set).  Recreate it with the ctypes-based
# NTFF hook from trn_boot when possible; degrade to a None hook (trace
# skipped, run still works) otherwise.
try:
    import antenv.axon_hooks  # noqa: F401
except ImportError:
    try:
        import antenv

        _hook = None
        try:
            from trn_agent_boot.trn_boot import _ntff_profile_via_ctypes

            _hook = _ntff_profile_via_ctypes("/opt/axon/libaxon_pjrt.so")
        except Exception:  # noqa: BLE001
            pass
        _mod = types.ModuleType("antenv.axon_hooks")
        _mod._hook = _hook
        _mod.set_axon_ntff_profile_hook = lambda h: setattr(_mod, "_hook", h)
        _mod.get_axon_ntff_profile_hook = lambda: _mod._hook
        sys.modules["antenv.axon_hooks"] = _mod
        antenv.axon_hooks = _mod
    except Exception:  # noqa: BLE001
        pass

BF16 = mybir.dt.bfloat16
F32 = mybir.dt.float32
AF = mybir.ActivationFunctionType

B, S, D, H = 2, 2048, 1024, 16
DH = D // H            # 64
NCORES = 8
NH = 4                 # heads per core
DL = NH * DH           # 256 local head dims per core
KD = D // 128          # 8 contraction chunks for projections
SQ = S // 512          # 4 q tiles of 512
ST = S // 128          # 16 s tiles of 128
SCALE = DH ** -0.5     # 0.125
LAG = 3                # ctx trails scores by LAG k-tiles

_NC = None

TRACE = False
LAST_RESULTS = None
DEBUG = False


def _build_nc():
    nc = bacc.Bacc("TRN2", target_bir_lowering=False, debug=False,
                   num_devices=NCORES)

    xT_d = nc.dram_tensor("xT", [D, S], BF16, kind="ExternalInput")
    wq_d = nc.dram_tensor("wqT", [D, DL], BF16, kind="ExternalInput")
    wk_d = nc.dram_tensor("wkT", [D, DL], BF16, kind="ExternalInput")
    wv_d = nc.dram_tensor("wvT", [D, DL], BF16, kind="ExternalInput")
    wo_d = nc.dram_tensor("woT", [DL, D], BF16, kind="ExternalInput")
    # [128, 2, 128] lower-triangle (q >= k) mask, duplicated per head half
    tri_d = nc.dram_tensor("tri", [128, 256], BF16, kind="ExternalInput")
    # ones in column 0, zeros elsewhere (M=32 so strip rows are zero-filled)
    onescol_d = nc.dram_tensor("onescol", [128, 32], BF16, kind="ExternalInput")
    # denominator broadcast selector: row 0 -> ones at cols [0,64),
    # row 32 -> ones at cols [64,128); all other rows zero
    selb_d = nc.dram_tensor("selb", [33, 128], BF16, kind="ExternalInput")
    out_d = nc.dram_tensor("out", [S, D], F32, kind="ExternalOutput")
    dbg = {}
    if DEBUG:
        for j in (1, 2):
            for p in (0, 1):
                dbg[f"q4_{j}_{p}"] = nc.dram_tensor(
                    f"dbg_q4_{j}_{p}", [128, 512], F32, kind="ExternalOutput")
                dbg[f"inv2_{j}_{p}"] = nc.dram_tensor(
                    f"dbg_inv2_{j}_{p}", [2, 512], F32, kind="ExternalOutput")
                dbg[f"invb_{j}_{p}"] = nc.dram_tensor(
                    f"dbg_invb_{j}_{p}", [128, 512], F32, kind="ExternalOutput")
                dbg[f"ctxT_{j}_{p}"] = nc.dram_tensor(
                    f"dbg_ctxT_{j}_{p}", [128, 512], BF16, kind="ExternalOutput")
                dbg[f"ex_{j}_{p}"] = nc.dram_tensor(
                    f"dbg_ex_{j}_{p}", [128, 1024], BF16, kind="ExternalOutput")

    with tile.TileContext(nc) as tc:
        with (
            tc.tile_pool(name="consts", bufs=1) as consts,
            tc.tile_pool(name="xpool", bufs=1) as xpool,
            tc.tile_pool(name="qkpool", bufs=1) as qkpool,
            tc.tile_pool(name="vpool", bufs=1) as vpool,
            tc.tile_pool(name="cpool", bufs=1) as cpool,
            tc.tile_pool(name="exppool", bufs=8) as exppool,
            tc.tile_pool(name="addpool", bufs=3) as addpool,
            tc.tile_pool(name="accpool", bufs=2) as accpool,
            tc.tile_pool(name="smallpool", bufs=2) as smallpool,
            tc.tile_pool(name="outpool", bufs=6) as outpool,
            tc.tile_pool(name="spsum", bufs=2, space="PSUM") as spsum,
            tc.tile_pool(name="ctxpsum", bufs=2, space="PSUM") as ctxpsum,
            tc.tile_pool(name="mpsum", bufs=1, space="PSUM") as mpsum,
            tc.tile_pool(name="rpsum", bufs=1, space="PSUM") as rpsum,
        ):
            # ---- constants (order matters: wq half, x sq0 halves, wk, wv) ----
            # q/k weights split into kd halves so the first projection rider
            # only waits on a quarter of the weight DMA bytes
            wq_sb = [consts.tile([128, 4, DL], BF16, name=f"wq{i}")
                     for i in range(2)]
            wk_sb = [consts.tile([128, 4, DL], BF16, name=f"wk{i}")
                     for i in range(2)]
            wv_sb = consts.tile([128, KD, DL], BF16)
            wo_sb = consts.tile([128, 2, D], BF16)
            tri_sb = consts.tile([128, 256], BF16)
            onescol_sb = consts.tile([128, 32], BF16)
            selb_sb = consts.tile([33, 128], BF16)
            zb = consts.tile([128, 1], F32)

            def wqt(kd):
                return wq_sb[kd // 4][:, kd % 4, :]

            def wkt(kd):
                return wk_sb[kd // 4][:, kd % 4, :]

            wq_r = wq_d.rearrange("(kd p) j -> p kd j", p=128)
            wk_r = wk_d.rearrange("(kd p) j -> p kd j", p=128)

            # ---- persistent activations ----
            xt = [
                [xpool.tile([128, 512], BF16, name=f"xt_{kd}_{sq}",
                            tag=f"xt_{kd}_{sq}") for sq in range(SQ)]
                for kd in range(KD)
            ]
            qT = [
                [qkpool.tile([128, 512], BF16, name=f"qT_{m2}_{sq}",
                             tag=f"qT_{m2}_{sq}") for sq in range(SQ)]
                for m2 in range(2)
            ]
            kT = [
                [qkpool.tile([128, 512], BF16, name=f"kT_{m2}_{sq}",
                             tag=f"kT_{m2}_{sq}") for sq in range(SQ)]
                for m2 in range(2)
            ]
            vt = [
                vpool.tile([128, NH, DH], BF16, name=f"v_{st}", tag=f"v_{st}")
                for st in range(ST)
            ]
            # ctx tiles split into column halves so the output projection can
            # start as soon as the first normalize half lands
            ctxT = [
                [[cpool.tile([128, 256], BF16, name=f"ctxT_{kc}_{sq}_{hf}",
                             tag=f"ctxT_{kc}_{sq}_{hf}") for hf in range(2)]
                 for sq in range(SQ)]
                for kc in range(2)
            ]

            xT_r = xT_d.rearrange("(kd p) s -> kd p s", p=128)
            # DMA priority: first projection rider (q, kd 0-3) needs wq half
            # 0 + the first four x chunks of sq=0; everything else follows
            nc.sync.dma_start(wq_sb[0][:], wq_r[:, 0:4, :])
            for kd in range(4):
                nc.sync.dma_start(xt[kd][0][:], xT_r[kd, :, 0:512])
            nc.sync.dma_start(wq_sb[1][:], wq_r[:, 4:8, :])
            for kd in range(4, KD):
                nc.sync.dma_start(xt[kd][0][:], xT_r[kd, :, 0:512])
            nc.sync.dma_start(wk_sb[0][:], wk_r[:, 0:4, :])
            nc.sync.dma_start(wk_sb[1][:], wk_r[:, 4:8, :])
            nc.sync.dma_start(tri_sb[:], tri_d[:])
            nc.sync.dma_start(onescol_sb[:], onescol_d[:])
            nc.vector.memset(zb[:], 0.0)
            nc.sync.dma_start(
                wv_sb[:], wv_d.rearrange("(kd p) j -> p kd j", p=128))
            nc.sync.dma_start(selb_sb[:], selb_d[:])
            for sq in range(1, SQ):
                for kd in range(KD):
                    nc.sync.dma_start(
                        xt[kd][sq][:], xT_r[kd, :, sq * 512:(sq + 1) * 512]
                    )
                if sq == 1:
                    nc.sync.dma_start(
                        wo_sb[:], wo_d.rearrange("(kc p) o -> p kc o", p=128))

            # ---- work emitters ----
            # riders alternate between the two single-buffer psum banks so a
            # rider's matmul never waits on the previous rider's evacuation
            _rps = {"i": 0}

            def rider_ps(cols=512):
                pl, tg = ((rpsum, "r"), (mpsum, "m"))[_rps["i"] % 2]
                _rps["i"] += 1
                return pl.tile([128, cols], F32, name="r_ps", tag=tg)

            def emit_qk_group(sq, which, m2):
                wt = wqt if which == "q" else wkt
                dst = qT if which == "q" else kT
                ps = rider_ps()
                for kd in range(KD):
                    nc.tensor.matmul(
                        ps[:],
                        wt(kd)[:, m2 * 128:(m2 + 1) * 128],
                        xt[kd][sq][:],
                        start=(kd == 0),
                        stop=(kd == KD - 1),
                    )
                nc.vector.tensor_copy(dst[m2][sq][:], ps[:])

            def qk_half_riders(sq, which, m2):
                # one q/k projection group split into two riders (4 of the 8
                # contraction chunks each) so a single slot's PE burst stays
                # short; the PSUM accumulator is shared via the closure
                wt = wqt if which == "q" else wkt
                dst = qT if which == "q" else kT
                st = {}

                def h1():
                    ps = rider_ps()
                    for kd in range(4):
                        nc.tensor.matmul(
                            ps[:],
                            wt(kd)[:, m2 * 128:(m2 + 1) * 128],
                            xt[kd][sq][:],
                            start=(kd == 0),
                            stop=False,
                        )
                    st["ps"] = ps

                def h2():
                    ps = st["ps"]
                    for kd in range(4, KD):
                        nc.tensor.matmul(
                            ps[:],
                            wt(kd)[:, m2 * 128:(m2 + 1) * 128],
                            xt[kd][sq][:],
                            start=False,
                            stop=(kd == KD - 1),
                        )
                    nc.vector.tensor_copy(dst[m2][sq][:], ps[:])

                return [h1, h2]

            def emit_v_group(sq, sti):
                st = sq * 4 + sti
                ps = rider_ps(DL)
                for kd in range(KD):
                    nc.tensor.matmul(
                        ps[:],
                        xt[kd][sq][:, sti * 128:(sti + 1) * 128],
                        wv_sb[:, kd, :],
                        start=(kd == 0),
                        stop=(kd == KD - 1),
                    )
                # split the v evacs between ScalarE and DVE (the (j,1)
                # loops these ride are paced by whichever engine is fuller)
                if sti % 2 == 0:
                    nc.scalar.copy(
                        vt[st][:].rearrange("p h d -> p (h d)"), ps[:]
                    )
                else:
                    nc.vector.tensor_copy(
                        vt[st][:].rearrange("p h d -> p (h d)"), ps[:]
                    )

            def emit_wo(j, sti, ot, pool=None, tag=None):
                st = j * 4 + sti
                if pool is None:
                    o_ps = rider_ps()
                else:
                    o_ps = pool.tile([128, 512], F32, name="r_ps", tag=tag)
                for kc in range(2):
                    nc.tensor.matmul(
                        o_ps[:],
                        ctxT[kc][j][sti // 2][:, (sti % 2) * 128:
                                              (sti % 2) * 128 + 128],
                        wo_sb[:, kc, ot * 512:(ot + 1) * 512],
                        start=(kc == 0),
                        stop=(kc == 1),
                    )
                ob = outpool.tile([128, 512], F32, name="ob", tag="ob")
                # alternate the PSUM evac between ScalarE (activation Copy,
                # same act table as Exp) and DVE so neither engine eats the
                # whole 26us; in the LATE loops (these riders run in (j+1,*))
                # ScalarE's exp cadence is the pacer, so keep it clean there
                if (j < 2 or j == SQ - 1) and (sti + ot) % 2 == 0:
                    nc.scalar.copy(ob[:], o_ps[:])
                else:
                    nc.vector.tensor_copy(ob[:], o_ps[:])
                nc.sync.dma_start(
                    out_d[st * 128:(st + 1) * 128, ot * 512:(ot + 1) * 512],
                    ob[:],
                )

            # normalize chain state per (j, p): set at loop end, consumed by
            # riders in the following loop.  q33 holds the per-head exp sums
            # at partitions 0 (head even) and 32 (head odd); selb broadcasts
            # row 0 to out cols [0,64) and row 32 to [64,128), then the
            # reciprocal runs on the already-broadcast (128, 512).
            def make_chain(j, p, q33, c_ps):
                st = {}

                def s_bcast():
                    denb = rider_ps()
                    nc.tensor.matmul(denb[:], selb_sb[:], q33[:],
                                     start=True, stop=True)
                    st["denb"] = denb

                def s_recip():
                    invb = smallpool.tile([128, 512], F32, name="invb",
                                          tag="invb")
                    nc.vector.reciprocal_approx_fast(invb[:], st["denb"][:])
                    st["invb"] = invb
                    if DEBUG and f"invb_{j}_{p}" in dbg:
                        nc.sync.dma_start(dbg[f"invb_{j}_{p}"][:], invb[:])

                def s_mul(hf):
                    nc.vector.tensor_mul(
                        ctxT[p][j][hf][:],
                        c_ps[:, hf * 256:hf * 256 + 256],
                        st["invb"][:, hf * 256:hf * 256 + 256],
                    )

                return [s_bcast, s_recip,
                        lambda: s_mul(0), lambda: s_mul(1)]

            # ---- attention loop for one (j, p) ----
            def attention(j, p, early, bulk, after=[]):
                nkt = 4 * j + 4
                # `early` chain riders run one-per-slot from slot 0 (they
                # recycle the ctx/m PSUM tiles); `bulk` riders (independent of
                # the chain) spread evenly over the loop; `after` riders (the
                # previous q tile's Wo, which READS what the chain writes)
                # must be emitted strictly after the last chain rider.
                E = len(early)
                L = len(bulk)
                A = len(after)
                rem = max(1, nkt - E)
                sched = [
                    ([early[kt]] if kt < E else [])
                    + bulk[(kt * L) // nkt:((kt + 1) * L) // nkt]
                    + (after[((kt - E) * A) // rem:((kt - E + 1) * A) // rem]
                       if kt >= E else [])
                    for kt in range(nkt)
                ]
                c_ps = ctxpsum.tile([128, 512], F32, name="c_ps", tag="ctx")
                exs = {}
                # running bf16 accumulator of all exp tiles (both heads);
                # the PE only sees it once, at loop end
                acc = accpool.tile([128, 1024], BF16, name="acc", tag="acc")
                acc3 = acc[:].rearrange("p (h q) -> p h q", h=2)

                def emit_scores_exp(kt):
                    o = kt - 4 * j
                    q0 = 128 * o if o > 0 else 0
                    s_ps = spsum.tile([128, 1024], F32, name="s_ps", tag="s")
                    for i2 in range(2):
                        hr = i2 * 64
                        nc.tensor.matmul(
                            s_ps[:, i2 * 512 + q0:(i2 + 1) * 512],
                            kT[p][kt // 4][hr:hr + 64,
                                           (kt % 4) * 128:(kt % 4 + 1) * 128],
                            qT[p][j][hr:hr + 64, q0:512],
                            start=True,
                            stop=True,
                        )
                    ex = exppool.tile([128, 1024], BF16, name="ex", tag="ex")
                    if q0 == 0:
                        nc.scalar.activation(
                            ex[:], s_ps[:], AF.Exp, bias=zb[:], scale=SCALE
                        )
                    else:
                        e3 = ex[:].rearrange("p (h q) -> p h q", h=2)
                        s3 = s_ps[:].rearrange("p (h q) -> p h q", h=2)
                        nc.scalar.activation(
                            e3[:, :, q0:512], s3[:, :, q0:512], AF.Exp,
                            bias=zb[:], scale=SCALE
                        )
                    if o >= 0:
                        # triangle mask on the 128-wide diagonal window
                        e3 = ex[:].rearrange("p (h q) -> p h q", h=2)
                        t3 = tri_sb[:].rearrange("p (h q) -> p h q", h=2)
                        nc.vector.tensor_mul(
                            e3[:, :, q0:q0 + 128], e3[:, :, q0:q0 + 128],
                            t3[:]
                        )
                    exs[kt] = ex
                    if DEBUG and kt == 4 * j + 1 and f"ex_{j}_{p}" in dbg:
                        nc.sync.dma_start(dbg[f"ex_{j}_{p}"][:], ex[:])

                def emit_ctx(kt):
                    o = kt - 4 * j
                    q0 = 128 * o if o > 0 else 0
                    ex = exs[kt]
                    e3 = ex[:].rearrange("p (h q) -> p h q", h=2)
                    for i2 in range(2):
                        nc.tensor.matmul(
                            c_ps[64 * i2:64 * i2 + DH, q0:512],
                            vt[kt][:, 2 * p + i2, :],
                            e3[:, i2, q0:512],
                            start=(kt == 0),
                            stop=(kt == nkt - 1),
                            tile_position=(0, 64 * i2),
                        )
                    if kt % 2 == 1:
                        # fold the (kt-1, kt) exp pair into the running
                        # denominator accumulator on DVE (pair sum, then
                        # in-place accumulate over the pair's valid range).
                        # The LAST pair skips the accumulate: its pair sum
                        # goes to the PE directly as a second denominator
                        # matmul, so the loop-end matmul never waits on the
                        # tail of the DVE add chain.
                        op = kt - 1 - 4 * j
                        qp = 128 * op if op > 0 else 0
                        exprev = exs.pop(kt - 1)
                        ep3 = exprev[:].rearrange("p (h q) -> p h q", h=2)
                        if q0 > qp:
                            # cur tile never wrote [qp, q0); zero it so the
                            # union-range add reads defined data
                            nc.gpsimd.memset(e3[:, :, qp:q0], 0.0)
                        if kt == 1:
                            nc.vector.tensor_add(acc[:], exprev[:], ex[:])
                        else:
                            es = addpool.tile([128, 1024], BF16, name="es",
                                              tag="es")
                            es3 = es[:].rearrange("p (h q) -> p h q", h=2)
                            nc.vector.tensor_add(
                                es3[:, :, qp:512], ep3[:, :, qp:512],
                                e3[:, :, qp:512]
                            )
                            if kt == nkt - 1 and nkt > 4:
                                mstate["last"] = (es3, qp)
                            else:
                                nc.vector.tensor_add(
                                    acc3[:, :, qp:512], acc3[:, :, qp:512],
                                    es3[:, :, qp:512]
                                )

                mstate = {"last": None}
                for kt in range(nkt):
                    # ctx(kt-LAG) is guaranteed-ready (its exp finished slots
                    # ago) — emit it ahead of the scores pair, which may wait
                    # on the s_ps ring, so the PE FIFO head never blocks idle
                    if kt >= LAG:
                        emit_ctx(kt - LAG)
                    emit_scores_exp(kt)
                    for r in sched[kt]:
                        r()
                for kt in range(max(0, nkt - LAG), nkt):
                    emit_ctx(kt)
                exs.clear()

                # single denominator reduction at loop end: one onescol
                # matmul chain per head into 32-row strips (accumulator +
                # the last pair sum), then a (33, 512) evac
                m_ps = mpsum.tile([128, 512], F32, name="m_ps", tag="m")
                for h in range(2):
                    nc.tensor.matmul(
                        m_ps[32 * h:32 * h + 32, :],
                        onescol_sb[:],
                        acc3[:, h, :],
                        start=True,
                        stop=(mstate["last"] is None),
                        tile_position=(0, 32 * h),
                    )
                if mstate["last"] is not None:
                    les3, lqp = mstate["last"]
                    for h in range(2):
                        nc.tensor.matmul(
                            m_ps[32 * h:32 * h + 32, lqp:512],
                            onescol_sb[:],
                            les3[:, h, lqp:512],
                            start=False,
                            stop=True,
                            tile_position=(0, 32 * h),
                        )
                q33 = smallpool.tile([33, 512], BF16, name="q33", tag="q33")
                if (j, p) == (SQ - 1, 1):
                    # last loop: DVE still drains the tail adds; ScalarE is
                    # free once the final exp lands, so the epilogue chain
                    # (bcast -> recip -> muls -> wo) starts sooner
                    nc.scalar.copy(q33[:], m_ps[0:33, :])
                else:
                    nc.vector.tensor_copy(q33[:], m_ps[0:33, :])
                if DEBUG and f"q4_{j}_{p}" in dbg:
                    nc.sync.dma_start(dbg[f"q4_{j}_{p}"][0:33, :], q33[:])
                return make_chain(j, p, q33, c_ps)

            # ---- prologue: just enough to start (j=0, p=0); half-riders so
            # the first matmul only waits on wq half 0 + x chunks 0-3 ----
            for r in qk_half_riders(0, "q", 0):
                r()
            for r in qk_half_riders(0, "k", 0):
                r()
            emit_v_group(0, 0)
            emit_v_group(0, 1)

            def qkv_riders(sq):
                return [
                    lambda s=sq: emit_qk_group(s, "q", 0),
                    lambda s=sq: emit_qk_group(s, "k", 0),
                    lambda s=sq: emit_qk_group(s, "q", 1),
                    lambda s=sq: emit_qk_group(s, "k", 1),
                    lambda s=sq: emit_v_group(s, 0),
                    lambda s=sq: emit_v_group(s, 1),
                    lambda s=sq: emit_v_group(s, 2),
                    lambda s=sq: emit_v_group(s, 3),
                ]

            def wo_riders(j):
                return [
                    (lambda jj=j, s=sti, o=ot: emit_wo(jj, s, o))
                    for sti in range(4) for ot in range(2)
                ]

            # ---- main loops ----
            chain = {}
            chain[(0, 0)] = attention(
                0, 0, [],
                [lambda: emit_qk_group(0, "q", 1),
                 lambda: emit_qk_group(0, "k", 1),
                 lambda: emit_v_group(0, 2),
                 lambda: emit_v_group(0, 3)],
            )
            chain[(0, 1)] = attention(0, 1, chain[(0, 0)], qkv_riders(1))
            # sq=3's k/v projections ride j=3's own (ScalarE-bound) loops —
            # they are only consumed from kt=12 — so PE-bound j=2 stays lean
            for j in range(1, SQ):
                # q/k of the next q tile ride (j,0); its v groups ride (j,1).
                # wo(j-1) riders split 4+4 across (j,0) and (j,1) so their
                # PSUM evacuations don't pile onto one loop's Scalar/DVE
                # budget (the exp cadence paces the late loops)
                qkv = qkv_riders(j + 1) if j + 1 < SQ else []
                wos = wo_riders(j - 1)
                chain[(j, 0)] = attention(j, 0, chain[(j - 1, 1)], qkv[:4],
                                          after=wos[:4])
                chain[(j, 1)] = attention(j, 1, chain[(j, 0)],
                                          qkv[4:] + wos[4:])

            # ---- epilogue ----
            for step in chain[(SQ - 1, 1)]:
                step()
            # rotate the last Wo tiles across the (now idle) PSUM rings so
            # they pipeline instead of serializing on the single rider bank
            epools = [(rpsum, "r"), (mpsum, "m"), (spsum, "s")]
            ei = 0
            for sti in range(4):
                for ot in range(2):
                    p, t = epools[ei % 3]
                    ei += 1
                    emit_wo(SQ - 1, sti, ot, pool=p, tag=t)

    nc.compile()
    return nc


def _get_nc():
    global _NC
    if _NC is None:
        _NC = _build_nc()
    return _NC


def _bf16(a):
    return np.ascontiguousarray(a).astype(ml_dtypes.bfloat16)


def kernel(x, Wq, Wk, Wv, Wo, bo):
    global LAST_RESULTS
    x = np.asarray(x, dtype=np.float32)
    Wq = np.asarray(Wq, dtype=np.float32)
    Wk = np.asarray(Wk, dtype=np.float32)
    Wv = np.asarray(Wv, dtype=np.float32)
    Wo = np.asarray(Wo, dtype=np.float32)
    bo = np.asarray(bo, dtype=np.float32)

    xT = [_bf16(x[b].T) for b in range(B)]          # (D, S)
    WqT = np.ascontiguousarray(Wq.T)                # (D, D): col slice = head rows
    WkT = np.ascontiguousarray(Wk.T)
    WvT = np.ascontiguousarray(Wv.T)
    WoT = np.ascontiguousarray(Wo.T)                # (D, D): row slice = ctx dims

    kk = np.arange(128)[:, None]
    cc = np.arange(128)[None, :]
    tri1 = (cc >= kk).astype(np.float32)            # (128, 128)
    tri = np.concatenate([tri1, tri1], axis=1).astype(ml_dtypes.bfloat16)

    onescol = np.zeros((128, 32), dtype=np.float32)
    onescol[:, 0] = 1.0
    onescol = onescol.astype(ml_dtypes.bfloat16)
    selb = np.zeros((33, 128), dtype=np.float32)
    selb[0, 0:64] = 1.0                             # head-even denom row
    selb[32, 64:128] = 1.0                          # head-odd denom row
    selb = selb.astype(ml_dtypes.bfloat16)

    in_maps = []
    for c in range(NCORES):
        b, g = divmod(c, 4)
        sl = slice(g * DL, (g + 1) * DL)
        in_maps.append(
            {
                "xT": xT[b],
                "wqT": _bf16(WqT[:, sl]),
                "wkT": _bf16(WkT[:, sl]),
                "wvT": _bf16(WvT[:, sl]),
                "woT": _bf16(WoT[sl, :]),
                "tri": tri,
                "onescol": onescol,
                "selb": selb,
            }
        )

    nc = _get_nc()
    results = run_bass_kernel_spmd(
        nc, in_maps, core_ids=list(range(NCORES)), trace=TRACE
    )
    LAST_RESULTS = results

    out = np.zeros((B, S, D), dtype=np.float32)
    for c in range(NCORES):
        out[c // 4] += results.results[c]["out"]
    out += bo[None, None, :]
    return out

